# revision 5
# baseline (speedup 1.0000x reference)
"""Trainium2 Bass kernel for nn_Contrast_54631984005844.

Strategy (8 NeuronCores, SPMD, two launches, row-sharded):

Launch 1 (projection + pair-loss matmul): core k owns rows R_k.
  - proj: y = proj(z) for the four z tensors on its row block, features on
    partitions. Both MLP layers run fp8 DoubleRow. ELU+1 is computed with a
    single DVE op via  elu(x)+1 = max(x+1, min(exp(x),1))  (e^x >= 1+x);
    the (b1+1) bias enters the PSUM through a rank-1 ones matmul, so
    stage-1 elementwise work is just ACT exp + DVE stt. Row norms^2 are
    reduced on GPSIMD and shipped to the host; normalization is deferred
    (host normalizes the moving side, the stationary side's 1/(tau*|y|)
    folds into launch-2's exp scale AP).
  - C = A @ B.T row block (A = z_mp1+z_sc1, B = z_mp2+z_sc2) in fp8
    DoubleRow, copies to bf16 split across ACT/DVE. Host gathers the
    2x67000 indexed entries and applies log-sigmoid.

Launch 2 (similarity): core k computes its RB x N row block of the two sim
  matrices in fp8 DoubleRow; ACT exp applies the per-row 1/(tau*|y_i|)
  scale (per-partition AP) with accum_out giving row sums (rsum); m is
  materialized to DRAM in fp8 (host gathers the ~21K pos-masked entries
  for num/numt — pos is 99.87% zeros, so pos never ships to the device);
  column-sum partials via DVE pair adds + GPSIMD partition_all_reduce.

Final scalar assembly on host in f64.
"""

import numpy as np
import ml_dtypes

import concourse.bass as bass
import concourse.mybir as mybir
import concourse.tile as tile
from concourse import bacc
from concourse import bass_isa
from concourse.bass_utils import run_bass_kernel_spmd

BF16 = mybir.dt.bfloat16
FP8 = mybir.dt.float8e4
F32 = mybir.dt.float32
AF = mybir.ActivationFunctionType
ALU = mybir.AluOpType
RED = bass_isa.ReduceOp
DR = mybir.MatmulPerfMode.DoubleRow

NPBF16 = ml_dtypes.bfloat16
NPFP8 = ml_dtypes.float8_e4m3

N = 4096          # rows per view
H = 512           # hidden dim
NC = 8            # cores
RB = N // NC      # row block = 512
TAU = 0.8
LAM = 0.5
INV_TAU = 1.0 / TAU
KC = H // 128     # 4 contraction chunks
AT = RB // 128    # 4 a-tiles

_CACHE = {}


# --------------------------------------------------------------------------
# Launch 1: projection (+ C = A @ B.T)
# --------------------------------------------------------------------------

def _build_l1(reps=None):
    nc = bacc.Bacc(None, target_bir_lowering=False, debug=False)

    zt_d = nc.declare_dram_parameter("zt", [4, H, RB], FP8, isOutput=False)
    w1t_d = nc.declare_dram_parameter("w1t", [H, H], FP8, isOutput=False)
    w2t_d = nc.declare_dram_parameter("w2t", [H, H], FP8, isOutput=False)
    b1p1_d = nc.declare_dram_parameter("b1p1", [1, H], BF16, isOutput=False)
    b2a_d = nc.declare_dram_parameter("b2a", [H, 1], F32, isOutput=False)
    atb_d = nc.declare_dram_parameter("atb", [H, RB], FP8, isOutput=False)
    btf_d = nc.declare_dram_parameter("btf", [H, N], FP8, isOutput=False)

    projT_d = nc.declare_dram_parameter("projT", [4, H, RB], BF16, isOutput=True)
    nrm_d = nc.declare_dram_parameter("nrm", [4, RB], F32, isOutput=True)
    c_d = nc.declare_dram_parameter("c", [RB, N], BF16, isOutput=True)

    with tile.TileContext(nc) as tc:
        with (
            tc.tile_pool(name="const", bufs=1) as cpool,
            tc.tile_pool(name="sb", bufs=1) as sb,
            tc.tile_pool(name="work", bufs=3) as work,
            tc.tile_pool(name="nrm", bufs=2) as nrmp,
            tc.tile_pool(name="cout", bufs=3) as coutp,
            tc.tile_pool(name="ps1", bufs=2, space="PSUM") as ps1p,
            tc.tile_pool(name="ps2", bufs=2, space="PSUM") as ps2p,
            tc.tile_pool(name="psc", bufs=2, space="PSUM") as pscp,
        ):
            def body():
                w1sb = cpool.tile([128, KC, H], FP8, tag="w1", name="w1sb")
                w2sb = cpool.tile([128, KC, H], FP8, tag="w2", name="w2sb")
                nc.sync.dma_start(w1sb[:], w1t_d[:].rearrange("(a p) o -> p a o", p=128))
                nc.sync.dma_start(w2sb[:], w2t_d[:].rearrange("(a p) o -> p a o", p=128))
                b1p1 = cpool.tile([1, H], BF16, tag="b1p1", name="b1p1")
                nc.sync.dma_start(b1p1[:], b1p1_d[:])
                b2sb = cpool.tile([128, KC], F32, tag="b2", name="b2sb")
                nc.sync.dma_start(b2sb[:], b2a_d[:].rearrange("(a p) one -> p (a one)", p=128))
                ones = cpool.tile([1, RB], BF16, tag="ones", name="ones")
                nc.vector.memset(ones[:], 1.0)
                neg1 = cpool.tile([128, 1], F32, tag="neg1", name="neg1")
                nc.vector.memset(neg1[:], -1.0)

                zts = []
                for t in range(4):
                    z = sb.tile([128, KC, RB], FP8, tag=f"zt{t}", name=f"zt{t}")
                    nc.sync.dma_start(z[:], zt_d[t].rearrange("(a p) r -> p a r", p=128))
                    zts.append(z)

                atbs = cpool.tile([128, KC, RB], FP8, tag="atb", name="atbs")
                nc.sync.dma_start(atbs[:], atb_d[:].rearrange("(a p) r -> p a r", p=128))
                btfs = sb.tile([128, KC, N], FP8, tag="btf", name="btfs")
                nc.sync.dma_start(btfs[:], btf_d[:].rearrange("(a p) b -> p a b", p=128))

                h1 = [sb.tile([128, KC, RB], FP8, tag=f"h1{t}", name=f"h1{t}")
                      for t in range(4)]
                yb = [sb.tile([128, KC, RB], BF16, tag=f"yb{t}", name=f"yb{t}")
                      for t in range(4)]
                sq = [sb.tile([128, KC, RB], BF16, tag=f"sq{t}", name=f"sq{t}")
                      for t in range(4)]

                def s1(t):
                    # psum = z @ W1.T + (b1+1);  h1 = max(psum, min(exp(psum-1),1))
                    for oc in range(KC):
                        p = ps1p.tile([128, RB], F32, tag="p1", name="p1")
                        for kp in (0, 2):
                            nc.tensor.matmul(
                                p[:], w1sb[:, kp:kp + 2, oc * 128:(oc + 1) * 128],
                                zts[t][:, kp:kp + 2, :],
                                start=(kp == 0), stop=False, perf_mode=DR)
                        nc.tensor.matmul(
                            p[:], b1p1[0:1, oc * 128:(oc + 1) * 128], ones[0:1, :],
                            start=False, stop=True, skip_group_check=True)
                        ex = work.tile([128, RB], BF16, tag="ex", name="ex")
                        nc.scalar.activation(ex[:], p[:], AF.Exp, bias=neg1[:])
                        nc.vector.scalar_tensor_tensor(
                            h1[t][:, oc, :], ex[:], 1.0, p[:], ALU.min, ALU.max)

                def s2(t):
                    # y = h1 @ W2.T + b2a ; squares ; ship y + later norms
                    for oc in range(KC):
                        p = ps2p.tile([128, RB], F32, tag="p2", name="p2")
                        for kp in (0, 2):
                            nc.tensor.matmul(
                                p[:], w2sb[:, kp:kp + 2, oc * 128:(oc + 1) * 128],
                                h1[t][:, kp:kp + 2, :],
                                start=(kp == 0), stop=(kp == 2), perf_mode=DR)
                        nc.scalar.activation(yb[t][:, oc, :], p[:], AF.Identity,
                                             bias=b2sb[:, oc:oc + 1])
                        nc.vector.tensor_tensor(sq[t][:, oc, :], yb[t][:, oc, :],
                                                yb[t][:, oc, :], ALU.mult)
                    nc.sync.dma_start(projT_d[t].rearrange("(a p) r -> p a r", p=128),
                                      yb[t][:])

                def fin(t):
                    n01 = nrmp.tile([128, RB], BF16, tag="n01", name="n01")
                    nc.vector.tensor_tensor(n01[:], sq[t][:, 0, :], sq[t][:, 1, :],
                                            ALU.add)
                    n23 = nrmp.tile([128, RB], BF16, tag="n23", name="n23")
                    nc.vector.tensor_tensor(n23[:], sq[t][:, 2, :], sq[t][:, 3, :],
                                            ALU.add)
                    nall = nrmp.tile([128, RB], BF16, tag="nall", name="nall")
                    nc.vector.tensor_tensor(nall[:], n01[:], n23[:], ALU.add)
                    nf = nrmp.tile([128, RB], F32, tag="nf", name="nf")
                    nc.gpsimd.partition_all_reduce(nf[:], nall[:], 128, RED.add)
                    nc.sync.dma_start(nrm_d[t:t + 1, :], nf[0:1, :])

                def cchunk(a, cw):
                    p = pscp.tile([128, 1024], F32, tag="pc", name="pc")
                    for kp in (0, 2):
                        for sub in range(2):
                            off = cw * 1024 + sub * 512
                            nc.tensor.matmul(
                                p[:, sub * 512:(sub + 1) * 512],
                                atbs[:, kp:kp + 2, a * 128:(a + 1) * 128],
                                btfs[:, kp:kp + 2, off:off + 512],
                                start=(kp == 0), stop=(kp == 2), perf_mode=DR)
                    cb = coutp.tile([128, 1024], BF16, tag="cb", name="cb")
                    if (a * 4 + cw) % 2 == 0:
                        nc.scalar.activation(cb[:], p[:], AF.Copy)
                    else:
                        nc.vector.tensor_copy(cb[:], p[:])
                    nc.sync.dma_start(
                        c_d[a * 128:(a + 1) * 128, cw * 1024:(cw + 1) * 1024], cb[:])

                # software-pipelined emission
                s1(0); s1(1)
                s2(0); s1(2)
                fin(0); s2(1)
                for cw in range(4):
                    cchunk(0, cw)
                s1(3); fin(1); s2(2)
                for cw in range(4):
                    cchunk(1, cw)
                fin(2); s2(3)
                for cw in range(4):
                    cchunk(2, cw)
                fin(3)
                for cw in range(4):
                    cchunk(3, cw)

            if reps:
                with tc.For_i(0, reps, 1):
                    body()
            else:
                body()

    nc.finalize()
    return nc


# --------------------------------------------------------------------------
# Launch 2: similarity row blocks, m materialized, rsum/csum reduced
# --------------------------------------------------------------------------

def _build_l2(reps=None):
    nc = bacc.Bacc(None, target_bir_lowering=False, debug=False)

    lm1_d = nc.declare_dram_parameter("lm1", [H, RB], FP8, isOutput=False)
    lm2_d = nc.declare_dram_parameter("lm2", [H, RB], FP8, isOutput=False)
    r1_d = nc.declare_dram_parameter("r1", [H, N], FP8, isOutput=False)
    r2_d = nc.declare_dram_parameter("r2", [H, N], FP8, isOutput=False)
    scl_d = nc.declare_dram_parameter("scl", [2, RB], F32, isOutput=False)

    m_d = nc.declare_dram_parameter("m", [2, RB, N], FP8, isOutput=True)
    racc_d = nc.declare_dram_parameter("racc", [128, 16], F32, isOutput=True)
    csr_d = nc.declare_dram_parameter("csr", [4, 2048], F32, isOutput=True)

    with tile.TileContext(nc) as tc:
        with (
            tc.tile_pool(name="res", bufs=1) as res,
            tc.tile_pool(name="rfull", bufs=1) as rfp,
            tc.tile_pool(name="acc", bufs=1) as accp,
            tc.tile_pool(name="mg", bufs=2) as mgp,
            tc.tile_pool(name="adds", bufs=2) as addp,
            tc.tile_pool(name="red", bufs=2) as redp,
            tc.tile_pool(name="ps", bufs=2, space="PSUM") as ps,
        ):
            def body():
                lm1 = res.tile([128, KC, RB], FP8, tag="lm1", name="lm1")
                lm2 = res.tile([128, KC, RB], FP8, tag="lm2", name="lm2")
                nc.sync.dma_start(lm1[:], lm1_d[:].rearrange("(a p) r -> p a r", p=128))
                nc.sync.dma_start(lm2[:], lm2_d[:].rearrange("(a p) r -> p a r", p=128))
                scl = res.tile([128, 2, AT], F32, tag="scl", name="scl")
                nc.sync.dma_start(scl[:], scl_d[:].rearrange("v (a p) -> p v a", p=128))
                r1 = rfp.tile([128, KC, N], FP8, tag="r1", name="r1")
                r2 = rfp.tile([128, KC, N], FP8, tag="r2", name="r2")
                # split loads by half so first sims start earlier
                for hf in range(2):
                    cs = slice(hf * 2048, (hf + 1) * 2048)
                    nc.sync.dma_start(
                        r1[:, :, cs],
                        r1_d[:, cs].rearrange("(a p) b -> p a b", p=128))
                    nc.sync.dma_start(
                        r2[:, :, cs],
                        r2_d[:, cs].rearrange("(a p) b -> p a b", p=128))

                racc = accp.tile([128, 16], F32, tag="racc", name="racc")

                for v, (lm, rr) in enumerate(((lm1, r1), (lm2, r2))):
                    for hf in range(2):
                        mgt = mgp.tile([128, AT, 2048], FP8, tag="mg", name="mg")
                        for a in range(AT):
                            p = ps.tile([128, 2048], F32, tag="ps", name="ps")
                            for kp in (0, 2):
                                for cc in range(4):
                                    off = hf * 2048 + cc * 512
                                    nc.tensor.matmul(
                                        p[:, cc * 512:(cc + 1) * 512],
                                        lm[:, kp:kp + 2, a * 128:(a + 1) * 128],
                                        rr[:, kp:kp + 2, off:off + 512],
                                        start=(kp == 0), stop=(kp == 2),
                                        perf_mode=DR)
                            slot = v * 8 + a * 2 + hf
                            nc.scalar.activation(
                                mgt[:, a, :], p[:], AF.Exp,
                                scale=scl[:, v, a:a + 1],
                                accum_out=racc[:, slot:slot + 1])
                            nc.sync.dma_start(
                                m_d[v][a * 128:(a + 1) * 128,
                                       hf * 2048:(hf + 1) * 2048],
                                mgt[:, a, :])
                        # csum partials: pair adds on DVE, reduce on GPSIMD
                        t01 = addp.tile([128, 2048], F32, tag="t01", name="t01")
                        nc.vector.tensor_tensor(t01[:], mgt[:, 0, :], mgt[:, 1, :],
                                                ALU.add)
                        t23 = addp.tile([128, 2048], F32, tag="t23", name="t23")
                        nc.vector.tensor_tensor(t23[:], mgt[:, 2, :], mgt[:, 3, :],
                                                ALU.add)
                        tsum = addp.tile([128, 2048], F32, tag="tsum", name="tsum")
                        nc.vector.tensor_tensor(tsum[:], t01[:], t23[:], ALU.add)
                        red = redp.tile([128, 2048], F32, tag="red", name="red")
                        nc.gpsimd.partition_all_reduce(red[:], tsum[:], 128, RED.add)
                        idx = v * 2 + hf
                        nc.sync.dma_start(csr_d[idx:idx + 1, :], red[0:1, :])

                nc.sync.dma_start(racc_d[:], racc[:])

            if reps:
                with tc.For_i(0, reps, 1):
                    body()
            else:
                body()

    nc.finalize()
    return nc


# --------------------------------------------------------------------------
# Host orchestration
# --------------------------------------------------------------------------

def _get_programs():
    if "l1" not in _CACHE:
        _CACHE["l1"] = _build_l1()
    if "l2" not in _CACHE:
        _CACHE["l2"] = _build_l2()
    return _CACHE["l1"], _CACHE["l2"]


def _fp8(x):
    return np.ascontiguousarray(np.asarray(x).astype(NPFP8))


def _make_l1_inputs(z_mp1, z_sc1, z_mp2, z_sc2, W1, b1, W2, b2):
    zts = [np.asarray(z.T).astype(NPFP8) for z in (z_mp1, z_sc1, z_mp2, z_sc2)]
    w1t = _fp8(W1.T)
    w2t = _fp8(W2.T)
    b1p1 = np.ascontiguousarray((b1 + 1.0).reshape(1, H).astype(NPBF16))
    b2a = np.ascontiguousarray((b2 - W2.sum(axis=1)).reshape(H, 1),
                               dtype=np.float32)
    A = (z_mp1 + z_sc1).astype(np.float32)
    B = (z_mp2 + z_sc2).astype(np.float32)
    atbT = np.asarray(A.T).astype(NPFP8)
    btf = _fp8(B.T)
    in1 = []
    for k in range(NC):
        sl = slice(k * RB, (k + 1) * RB)
        zt = np.ascontiguousarray(np.stack([z[:, sl] for z in zts]))
        in1.append({"zt": zt, "w1t": w1t, "w2t": w2t, "b1p1": b1p1, "b2a": b2a,
                    "atb": np.ascontiguousarray(atbT[:, sl]), "btf": btf})
    return in1


def _make_l2_inputs(res1):
    # projT blocks hold RAW y.T (bf16); nrm holds |y|^2 rows.
    yT = [np.asarray(res1[k]["projT"], np.float32) for k in range(NC)]   # [4,H,RB]
    rn = [1.0 / np.sqrt(np.asarray(res1[k]["nrm"], np.float64)) for k in range(NC)]
    # moving side: normalized fp8, full width
    r1f = np.concatenate([yT[k][1] * rn[k][1][None, :] for k in range(NC)],
                         axis=1)
    r2f = np.concatenate([yT[k][3] * rn[k][3][None, :] for k in range(NC)],
                         axis=1)
    r1f = _fp8(r1f)
    r2f = _fp8(r2f)
    in2 = []
    for k in range(NC):
        scl = np.ascontiguousarray(
            np.stack([rn[k][0], rn[k][2]]).astype(np.float32) * INV_TAU)
        in2.append({
            "lm1": _fp8(yT[k][0]),
            "lm2": _fp8(yT[k][2]),
            "r1": r1f, "r2": r2f,
            "scl": scl,
        })
    return in2


def _finish(res1, res2, pos1, pos2, pos_i, pos_j, neg_i, neg_j):
    rsum = np.zeros((2, N), np.float64)
    csum = np.zeros((2, N), np.float64)
    for k in range(NC):
        racc = np.asarray(res2[k]["racc"], np.float64)  # [128, 16]
        csr = np.asarray(res2[k]["csr"], np.float64)    # [4, 2048]
        for v in range(2):
            for a in range(AT):
                rows = k * RB + a * 128 + np.arange(128)
                rsum[v, rows] = racc[:, v * 8 + a * 2] + racc[:, v * 8 + a * 2 + 1]
            for hf in range(2):
                csum[v, hf * 2048:(hf + 1) * 2048] += csr[v * 2 + hf]

    M1 = np.concatenate([np.asarray(res2[k]["m"][0]) for k in range(NC)],
                        axis=0).astype(np.float32)
    M2 = np.concatenate([np.asarray(res2[k]["m"][1]) for k in range(NC)],
                        axis=0).astype(np.float32)

    losses = []
    for v, (M, pos) in enumerate(((M1, pos1), (M2, pos2))):
        r, c = np.nonzero(pos > 0.5)
        num = np.bincount(r, weights=M[r, c].astype(np.float64), minlength=N)
        numt = np.bincount(r, weights=M[c, r].astype(np.float64), minlength=N)
        l_mp = -np.log(num / (rsum[v] + 1e-8)).mean()
        l_sc = -np.log(numt / (csum[v] + 1e-8)).mean()
        losses.append(LAM * l_mp + (1.0 - LAM) * l_sc)

    C = np.concatenate([np.asarray(res1[k]["c"]).astype(np.float32)
                        for k in range(NC)], axis=0)
    ip1 = C[pos_i, pos_j].astype(np.float64)
    ip2 = C[neg_i, neg_j].astype(np.float64)

    def logsig(x):
        return -np.logaddexp(0.0, -x)

    loss_main = -logsig(ip1).mean() + logsig(-ip2).mean()
    return np.float32(loss_main + losses[0] + losses[1])


def kernel(z_mp1, z_sc1, pos1, z_mp2, z_sc2, pos2,
           W1, b1, W2, b2, pos_i, pos_j, neg_i, neg_j):
    z_mp1 = np.asarray(z_mp1, np.float32)
    z_sc1 = np.asarray(z_sc1, np.float32)
    z_mp2 = np.asarray(z_mp2, np.float32)
    z_sc2 = np.asarray(z_sc2, np.float32)
    pos1 = np.asarray(pos1, np.float32)
    pos2 = np.asarray(pos2, np.float32)
    W1 = np.asarray(W1, np.float32)
    W2 = np.asarray(W2, np.float32)
    b1 = np.asarray(b1, np.float32)
    b2 = np.asarray(b2, np.float32)
    pos_i = np.asarray(pos_i)
    pos_j = np.asarray(pos_j)
    neg_i = np.asarray(neg_i)
    neg_j = np.asarray(neg_j)

    l1, l2 = _get_programs()
    cores = list(range(NC))

    in1 = _make_l1_inputs(z_mp1, z_sc1, z_mp2, z_sc2, W1, b1, W2, b2)
    res1 = run_bass_kernel_spmd(l1, in1, cores).results

    in2 = _make_l2_inputs(res1)
    res2 = run_bass_kernel_spmd(l2, in2, cores).results

    return _finish(res1, res2, pos1, pos2, pos_i, pos_j, neg_i, neg_j)
